# revision 22
# baseline (speedup 1.0000x reference)
"""Multi-head causal attention (B=4,T=2048,C=768,H=12,HS=64) on 8 trn2 cores.

Sharding: core c -> (batch b=c//2, head-half s=c%2, heads 6s..6s+5).
Each core computes QKV + attention for its 6 heads over the full T, then the
cores of a batch exchange attention outputs via a world AllGather and each
computes the full output projection for its batch.  Host keeps core 2b's
output for batch b.

All matmuls in bf16 (PSUM accumulates fp32).  Scores are computed
transposed ([tk, tq]) so softmax denominators come from a ones-column in V
and no PE transposes of the TxT attention matrix are needed.
"""
import sys

if "/opt/trn_rl_repo" not in sys.path:
    sys.path.insert(0, "/opt/trn_rl_repo")

import numpy as np
import ml_dtypes

import concourse.bass as bass
import concourse.mybir as mybir
import concourse.tile as tile
from concourse import bacc
from concourse.bass import ts, ds
from concourse.bass_utils import run_bass_kernel_spmd
from concourse.masks import make_identity, make_upper_triangular

B, T_FULL, C, H, HS = 4, 2048, 768, 12, 64
N_CORES = 8
P = 128
NCH = C // P  # 6 c-chunks
dt = mybir.dt
bf16 = ml_dtypes.bfloat16
F32 = dt.float32
BF = dt.bfloat16


def build(T=T_FULL):
    nq = T // 512          # number of 512-wide q ranges
    nkt = T // P           # number of 128-wide k tiles
    nc = bacc.Bacc("TRN2", target_bir_lowering=False, debug=False,
                   num_devices=N_CORES)
    xt_d = nc.dram_tensor("xt", [C, T], BF, kind="ExternalInput").ap()
    wq_d = nc.dram_tensor("wq", [C, 384], BF, kind="ExternalInput").ap()
    wk_d = nc.dram_tensor("wk", [C, 384], BF, kind="ExternalInput").ap()
    wv_d = nc.dram_tensor("wv", [C, 384], BF, kind="ExternalInput").ap()
    wpt_d = nc.dram_tensor("wpt", [C, C], BF, kind="ExternalInput").ap()
    bp_d = nc.dram_tensor("bp", [1, C], BF, kind="ExternalInput").ap()
    meta_d = nc.dram_tensor("meta", [64, 2], dt.int32, kind="ExternalInput").ap()
    out_d = nc.dram_tensor("out", [T, C], F32, kind="ExternalOutput").ap()

    with tile.TileContext(nc) as tc:
        with tc.tile_pool(name="const", bufs=1) as const, \
             tc.tile_pool(name="big", bufs=1) as big, \
             tc.tile_pool(name="sbp", bufs=2) as sbp, \
             tc.tile_pool(name="ptp", bufs=3) as ptp, \
             tc.tile_pool(name="pp", bufs=1, space="PSUM") as pp, \
             tc.tile_pool(name="psc", bufs=2, space="PSUM") as psc, \
             tc.tile_pool(name="pso", bufs=3, space="PSUM") as pso, \
             tc.tile_pool(name="dram", bufs=1, space="DRAM") as dram:

            # ---- persistent loads ----
            xt = big.tile([P, NCH, T], BF, tag="xt")
            nc.sync.dma_start(xt[:], xt_d.rearrange("(n p) m -> p n m", p=P))
            wq = big.tile([P, NCH, 3, P], BF, tag="wq")
            wk = big.tile([P, NCH, 3, P], BF, tag="wk")
            wv = big.tile([P, NCH, 3, P], BF, tag="wv")
            for wt, wd in ((wq, wq_d), (wk, wk_d), (wv, wv_d)):
                nc.scalar.dma_start(
                    wt[:], wd.rearrange("(n p) (r m) -> p n r m", p=P, m=P))
            wpt = big.tile([64, 12, C], BF, tag="wpt")
            bp_sb = const.tile([1, C], BF, tag="bp")
            meta_sb = const.tile([64, 2], dt.int32, tag="meta")

            def late_loads():
                nc.sync.dma_start(wpt[:],
                                  wpt_d.rearrange("(j p) m -> p j m", p=64))
                nc.sync.dma_start(bp_sb[:], bp_d[:])
                nc.sync.dma_start(meta_sb[:], meta_d[:])

            ones1 = const.tile([1, P], BF, tag="ones1")
            nc.gpsimd.memset(ones1[:], 1.0)
            ident = const.tile([P, P], BF, tag="ident")
            make_identity(nc, ident[:])
            tri = const.tile([P, P], BF, tag="tri")
            make_upper_triangular(nc, tri[:], val=1.0, diag=True)

            att_all = big.tile([64, 6, T], BF, tag="att_all")
            att_oth = big.tile([64, 6, T], BF, tag="att_oth")

            EXP = mybir.ActivationFunctionType.Exp

            # ---- per head-pair: QKV + attention ----
            # QKV matmuls for pair r+1 are interleaved into pair r's
            # attention loop as PE filler work so the tensor engine never
            # idles (keeps the HAM clock gate at 2.4 GHz).
            def alloc_pair_tiles(r):
                qt = sbp.tile([P, T], BF, tag="qt", name=f"qt{r}")
                kt_t = sbp.tile([P, T], BF, tag="kt", name=f"kt{r}")
                vt_t = sbp.tile([P, T], BF, tag="vt", name=f"vt{r}")
                vaug0 = sbp.tile([P, nkt, 128], BF, tag="vaug0",
                                 name=f"vaug0_{r}")
                vaug1 = sbp.tile([P, nkt, 128], BF, tag="vaug1",
                                 name=f"vaug1_{r}")
                nc.gpsimd.memset(vaug0[:, :, 64:128], 1.0)
                nc.gpsimd.memset(vaug1[:, :, 64:128], 1.0)
                return qt, kt_t, vt_t, vaug0, vaug1

            def qkv_units(r, tiles):
                """Yield closures, each emitting one PE work unit."""
                qt, kt_t, vt_t, vaug0, vaug1 = tiles

                def mk_proj(wt, dst, tr):
                    def emit():
                        ps = pp.tile([P, 512], F32, tag="qkv", name="qkvps")
                        for ci in range(NCH):
                            nc.tensor.matmul(ps[:], wt[:, ci, r, :],
                                             xt[:, ci, ts(tr, 512)],
                                             start=(ci == 0),
                                             stop=(ci == NCH - 1))
                        nc.vector.tensor_copy(dst[:, ts(tr, 512)], ps[:])
                    return emit

                for wt, dst in ((wq, qt), (wk, kt_t), (wv, vt_t)):
                    for tr in range(nq):
                        yield mk_proj(wt, dst, tr)

                def mk_tp(k4):
                    def emit():
                        for k in range(k4, min(k4 + 4, nkt)):
                            pst = pp.tile([P, P], BF, tag="qkv", name="pst")
                            nc.tensor.transpose(pst[:], vt_t[:, ts(k, P)],
                                                ident[:])
                            nc.vector.tensor_copy(vaug0[:, k, 0:64],
                                                  pst[:, 0:64])
                            nc.vector.tensor_copy(vaug1[:, k, 0:64],
                                                  pst[:, 64:128])
                    return emit

                for k4 in range(0, nkt, 4):
                    yield mk_tp(k4)

            # own-half projection accumulator; shares the xt slot (xt's last
            # use is pair 2's QKV, which precedes all pass-A writes)
            co_ranges = [(0, 512), (512, C)]

            def projA_unit(tch, acc):
                def emit():
                    for (c0, c1) in co_ranges:
                        ps1 = pp.tile([P, c1 - c0], F32, tag="qkv",
                                      name="pAps")
                        for j in range(6):
                            nc.tensor.matmul(ps1[:],
                                             att_all[:, j, ts(tch, P)],
                                             wpt[:, j, c0:c1],
                                             start=(j == 0), stop=(j == 5))
                        nc.vector.tensor_copy(acc[:, tch, c0:c1], ps1[:])
                return emit

            def emit_ag_half(r, h):
                Th = T // 2
                in_cc = dram.tile([P, Th], BF, tag=f"in_cc{r}_{h}",
                                  name=f"in_cc{r}_{h}")
                out_cc = dram.tile([N_CORES * P, Th], BF,
                                   tag=f"out_cc{r}_{h}", name=f"out_cc{r}_{h}")
                nc.sync.dma_start(
                    in_cc.rearrange("(p j) m -> p j m", j=2),
                    att_all[:, 2 * r:2 * r + 2, h * Th:(h + 1) * Th])
                nc.gpsimd.collective_compute(
                    "AllGather", mybir.AluOpType.bypass,
                    replica_groups=[list(range(N_CORES))],
                    ins=[in_cc.opt()], outs=[out_cc.opt()])
                for a in (0, 1):
                    nc.gpsimd.indirect_dma_start(
                        out=att_oth[:, 2 * r + a, h * Th:(h + 1) * Th],
                        out_offset=None,
                        in_=out_cc[:],
                        in_offset=bass.IndirectOffsetOnAxis(
                            ap=meta_sb[:, a:a + 1], axis=0),
                    )

            pair_tiles = alloc_pair_tiles(0)
            for u in qkv_units(0, pair_tiles):
                u()
            late_loads()
            acc = None

            for r in range(3):
                qt, kt_t, vt_t, vaug0, vaug1 = pair_tiles
                vaug = (vaug0, vaug1)
                if r < 2:
                    next_tiles = alloc_pair_tiles(r + 1)
                    fillers = list(qkv_units(r + 1, next_tiles))
                else:
                    next_tiles, fillers = [], []
                    acc = big.tile([P, T // P, C], F32, tag="xt", name="acc")
                n_iters = sum(4 * qr + 4 for qr in range(nq))
                fill_i, iter_i = 0, 0

                # attention per q-range; one sc/pt tile covers both heads of
                # the pair per k-tile so psc bufs=2 gives true lookahead
                for qr in range(nq):
                    n_k = 4 * qr + 4       # causal: k tiles 0..4qr+3
                    oT0 = pso.tile([P, 512], F32, tag="oT")
                    oT1 = pso.tile([P, 512], F32, tag="oT")
                    oT = (oT0, oT1)
                    for kti in range(n_k):
                        m = kti - 4 * qr
                        lo = 128 * max(0, m)  # causal col offset
                        sc = psc.tile([P, 2, 512], F32, tag="sc")
                        for a in (0, 1):
                            nc.tensor.matmul(
                                sc[:, a, lo:512],
                                kt_t[64 * a:64 * a + 64, ts(kti, P)],
                                qt[64 * a:64 * a + 64,
                                   qr * 512 + lo:(qr + 1) * 512],
                                start=True, stop=True)
                        iter_i += 1
                        while (fill_i < len(fillers)
                               and fill_i * n_iters < iter_i * len(fillers)):
                            fillers[fill_i]()
                            fill_i += 1
                        pt = ptp.tile([P, 2, 512], BF, tag="pt")
                        nc.scalar.activation(pt[:, :, lo:512],
                                             sc[:, :, lo:512], EXP)
                        if 0 <= m < 4:
                            for a in (0, 1):
                                blk = pt[:, a, 128 * m:128 * (m + 1)]
                                nc.vector.tensor_mul(blk, blk, tri[:])
                        for a in (0, 1):
                            nc.tensor.matmul(
                                oT[a][:, lo:512],
                                vaug[a][:, kti, :],
                                pt[:, a, lo:512],
                                start=(kti == 0), stop=(kti == n_k - 1))
                    for a in (0, 1):
                        rb = sbp.tile([64, 512], F32, tag="rb")
                        nc.vector.reciprocal(rb[:], oT[a][64:128, :])
                        nc.vector.tensor_mul(
                            att_all[:, 2 * r + a, ts(qr, 512)],
                            oT[a][0:64, :], rb[:])

                    if r == 2:
                        # own-half projection for this q-range's t-chunks
                        # doubles as PE filler for the last pair
                        for tch in range(4 * qr, min(4 * qr + 4, T // P)):
                            projA_unit(tch, acc)()
                    # first-half exchange can fire once cols 0..T/2 are done
                    if qr == nq // 2 - 1:
                        emit_ag_half(r, 0)

                while fill_i < len(fillers):
                    fillers[fill_i]()
                    fill_i += 1
                pair_tiles = next_tiles
                if nq < 2:
                    emit_ag_half(r, 0)
                emit_ag_half(r, 1)

            # ---- output projection pass B: partner half + bias ----
            for tch in range(T // P):
                po = psc.tile([P, C], F32, tag="sc")
                for j in range(6):
                    lhsT = att_oth[:, j, ts(tch, P)]
                    for (c0, c1) in co_ranges:
                        nc.tensor.matmul(po[:, c0:c1], lhsT,
                                         wpt[:, 6 + j, c0:c1],
                                         start=(j == 0), stop=False)
                for (c0, c1) in co_ranges:
                    nc.tensor.matmul(po[:, c0:c1], ones1[:],
                                     bp_sb[:, c0:c1], start=False, stop=True)
                ot = sbp.tile([P, C], F32, tag="out")
                nc.vector.tensor_add(ot[:], po[:], acc[:, tch, :])
                nc.sync.dma_start(out_d[ts(tch, P), :], ot[:])

    nc.compile()
    return nc


_cached = {}


def get_nc(T=T_FULL):
    if T not in _cached:
        _cached[T] = build(T)
    return _cached[T]


def _make_in_maps(x, Wq, Wk, Wv, Wp, bp):
    scale = HS ** -0.5
    in_maps = []
    for c in range(N_CORES):
        b, s = c // 2, c % 2
        heads = list(range(6 * s, 6 * s + 6))
        xt = np.ascontiguousarray(np.asarray(x)[b].T).astype(bf16)
        def packw(W, sc=1.0):
            cols = []
            for rr in range(3):
                h0, h1 = heads[2 * rr], heads[2 * rr + 1]
                cols.append(np.concatenate([W[h0], W[h1]], axis=1))
            return (np.concatenate(cols, axis=1) * sc).astype(bf16)
        order = heads + [h for h in range(H) if h not in heads]
        wpt = np.concatenate(
            [np.asarray(Wp)[:, 64 * h:64 * h + 64].T for h in order],
            axis=0).astype(bf16)
        meta = ((c ^ 1) * 128 + 2 * np.arange(64, dtype=np.int32)[:, None]
                + np.arange(2, dtype=np.int32)[None, :]).astype(np.int32)
        in_maps.append({
            "xt": xt,
            "wq": packw(np.asarray(Wq), scale),
            "wk": packw(np.asarray(Wk)),
            "wv": packw(np.asarray(Wv)),
            "wpt": wpt,
            "bp": np.asarray(bp).reshape(1, C).astype(bf16),
            "meta": meta,
        })
    return in_maps


def kernel(x, Wq, Wk, Wv, Wp, bp):
    nc = get_nc(T_FULL)
    in_maps = _make_in_maps(x, Wq, Wk, Wv, Wp, bp)
    res = run_bass_kernel_spmd(nc, in_maps, list(range(N_CORES)))
    out = np.stack([res.results[2 * b]["out"] for b in range(B)])
    return out.astype(np.float32)


# revision 28
# speedup vs baseline: 1.0685x; 1.0685x over previous
"""Multi-head causal attention (B=4,T=2048,C=768,H=12,HS=64) on 8 trn2 cores.

Sharding: core c -> (batch b=c//2, head-half s=c%2, heads 6s..6s+5).
Each core computes QKV + attention for its 6 heads over the full T, then the
cores of a batch exchange attention outputs via a world AllGather and each
computes the full output projection for its batch.  Host keeps core 2b's
output for batch b.

All matmuls in bf16 (PSUM accumulates fp32).  Scores are computed
transposed ([tk, tq]) so softmax denominators come from a ones-column in V
and no PE transposes of the TxT attention matrix are needed.
"""
import sys

if "/opt/trn_rl_repo" not in sys.path:
    sys.path.insert(0, "/opt/trn_rl_repo")

import numpy as np
import ml_dtypes

import concourse.bass as bass
import concourse.mybir as mybir
import concourse.tile as tile
from concourse import bacc
from concourse.bass import ts, ds
from concourse.bass_utils import run_bass_kernel_spmd
from concourse.masks import make_identity, make_upper_triangular

B, T_FULL, C, H, HS = 4, 2048, 768, 12, 64
N_CORES = 8
P = 128
NCH = C // P  # 6 c-chunks
dt = mybir.dt
bf16 = ml_dtypes.bfloat16
F32 = dt.float32
BF = dt.bfloat16


def build(T=T_FULL):
    nq = T // 512          # number of 512-wide q ranges
    nkt = T // P           # number of 128-wide k tiles
    nc = bacc.Bacc("TRN2", target_bir_lowering=False, debug=False,
                   num_devices=N_CORES)
    xt_d = nc.dram_tensor("xt", [C, T], BF, kind="ExternalInput").ap()
    wq_d = nc.dram_tensor("wq", [C, 384], BF, kind="ExternalInput").ap()
    wk_d = nc.dram_tensor("wk", [C, 384], BF, kind="ExternalInput").ap()
    wv_d = nc.dram_tensor("wv", [C, 384], BF, kind="ExternalInput").ap()
    wpt_d = nc.dram_tensor("wpt", [C, C], BF, kind="ExternalInput").ap()
    bp_d = nc.dram_tensor("bp", [1, C], BF, kind="ExternalInput").ap()
    meta_d = nc.dram_tensor("meta", [64, 2], dt.int32, kind="ExternalInput").ap()
    out_d = nc.dram_tensor("out", [T, C], F32, kind="ExternalOutput").ap()

    with tile.TileContext(nc) as tc:
        with tc.tile_pool(name="const", bufs=1) as const, \
             tc.tile_pool(name="big", bufs=1) as big, \
             tc.tile_pool(name="sbp", bufs=2) as sbp, \
             tc.tile_pool(name="ptp", bufs=2) as ptp, \
             tc.tile_pool(name="pp", bufs=2, space="PSUM") as pp, \
             tc.tile_pool(name="psc", bufs=2, space="PSUM") as psc, \
             tc.tile_pool(name="pso", bufs=2, space="PSUM") as pso, \
             tc.tile_pool(name="dram", bufs=1, space="DRAM") as dram:

            # ---- persistent loads ----
            xt = big.tile([P, NCH, T], BF, tag="xt")
            nc.sync.dma_start(xt[:], xt_d.rearrange("(n p) m -> p n m", p=P))
            wq = big.tile([P, NCH, 3, P], BF, tag="wq")
            wk = big.tile([P, NCH, 3, P], BF, tag="wk")
            wv = big.tile([P, NCH, 3, P], BF, tag="wv")
            for wt, wd in ((wq, wq_d), (wk, wk_d), (wv, wv_d)):
                nc.scalar.dma_start(
                    wt[:], wd.rearrange("(n p) (r m) -> p n r m", p=P, m=P))
            wpt = big.tile([64, 12, C], BF, tag="wpt")
            bp_sb = const.tile([1, C], BF, tag="bp")
            meta_sb = const.tile([64, 2], dt.int32, tag="meta")

            def late_loads():
                nc.sync.dma_start(wpt[:],
                                  wpt_d.rearrange("(j p) m -> p j m", p=64))
                nc.sync.dma_start(bp_sb[:], bp_d[:])
                nc.sync.dma_start(meta_sb[:], meta_d[:])

            ones1 = const.tile([1, P], BF, tag="ones1")
            nc.gpsimd.memset(ones1[:], 1.0)
            ident = const.tile([P, P], BF, tag="ident")
            make_identity(nc, ident[:])
            tri = const.tile([P, P], BF, tag="tri")
            make_upper_triangular(nc, tri[:], val=1.0, diag=True)

            att_all = big.tile([64, 6, T], BF, tag="att_all")
            att_oth = big.tile([64, 6, T], BF, tag="att_oth")

            EXP = mybir.ActivationFunctionType.Exp

            # ---- per head-pair: QKV + attention ----
            # QKV matmuls for pair r+1 are interleaved into pair r's
            # attention loop as PE filler work so the tensor engine never
            # idles (keeps the HAM clock gate at 2.4 GHz).
            def alloc_pair_tiles(r):
                qt = sbp.tile([P, T], BF, tag="qt", name=f"qt{r}")
                kt_t = sbp.tile([P, T], BF, tag="kt", name=f"kt{r}")
                vt_t = sbp.tile([P, T], BF, tag="vt", name=f"vt{r}")
                vaug0 = sbp.tile([P, nkt, 128], BF, tag="vaug0",
                                 name=f"vaug0_{r}")
                vaug1 = sbp.tile([P, nkt, 128], BF, tag="vaug1",
                                 name=f"vaug1_{r}")
                nc.gpsimd.memset(vaug0[:, :, 64:128], 1.0)
                nc.gpsimd.memset(vaug1[:, :, 64:128], 1.0)
                return qt, kt_t, vt_t, vaug0, vaug1

            def qkv_units(r, tiles):
                """Yield closures, each emitting one PE work unit."""
                qt, kt_t, vt_t, vaug0, vaug1 = tiles

                def mk_proj(wt, dst, tr):
                    def emit():
                        ps = pp.tile([P, 512], F32, tag="qkv", name="qkvps")
                        for ci in range(NCH):
                            nc.tensor.matmul(ps[:], wt[:, ci, r, :],
                                             xt[:, ci, ts(tr, 512)],
                                             start=(ci == 0),
                                             stop=(ci == NCH - 1))
                        nc.vector.tensor_copy(dst[:, ts(tr, 512)], ps[:])
                    return emit

                for wt, dst in ((wq, qt), (wk, kt_t), (wv, vt_t)):
                    for tr in range(nq):
                        yield mk_proj(wt, dst, tr)

                def mk_tp(k4):
                    def emit():
                        for k in range(k4, min(k4 + 4, nkt)):
                            pst = pp.tile([P, P], BF, tag="qkv", name="pst")
                            nc.tensor.transpose(pst[:], vt_t[:, ts(k, P)],
                                                ident[:])
                            nc.vector.tensor_copy(vaug0[:, k, 0:64],
                                                  pst[:, 0:64])
                            nc.vector.tensor_copy(vaug1[:, k, 0:64],
                                                  pst[:, 64:128])
                    return emit

                for k4 in range(0, nkt, 4):
                    yield mk_tp(k4)

            # own-half projection accumulator; shares the xt slot (xt's last
            # use is pair 2's QKV, which precedes all pass-A writes)
            co_ranges = [(0, 512), (512, C)]

            def projA_unit(tch, acc):
                def emit():
                    for (c0, c1) in co_ranges:
                        ps1 = pp.tile([P, c1 - c0], F32, tag="qkv",
                                      name="pAps")
                        for j in range(6):
                            nc.tensor.matmul(ps1[:],
                                             att_all[:, j, ts(tch, P)],
                                             wpt[:, j, c0:c1],
                                             start=(j == 0), stop=(j == 5))
                        nc.vector.tensor_copy(acc[:, tch, c0:c1], ps1[:])
                return emit

            def emit_ag_half(r, h):
                Th = T // 2
                in_cc = dram.tile([P, Th], BF, tag=f"in_cc{r}_{h}",
                                  name=f"in_cc{r}_{h}")
                out_cc = dram.tile([N_CORES * P, Th], BF,
                                   tag=f"out_cc{r}_{h}", name=f"out_cc{r}_{h}")
                nc.sync.dma_start(
                    in_cc.rearrange("(p j) m -> p j m", j=2),
                    att_all[:, 2 * r:2 * r + 2, h * Th:(h + 1) * Th])
                nc.gpsimd.collective_compute(
                    "AllGather", mybir.AluOpType.bypass,
                    replica_groups=[list(range(N_CORES))],
                    ins=[in_cc.opt()], outs=[out_cc.opt()])
                for a in (0, 1):
                    nc.gpsimd.indirect_dma_start(
                        out=att_oth[:, 2 * r + a, h * Th:(h + 1) * Th],
                        out_offset=None,
                        in_=out_cc[:],
                        in_offset=bass.IndirectOffsetOnAxis(
                            ap=meta_sb[:, a:a + 1], axis=0),
                    )

            pair_tiles = alloc_pair_tiles(0)
            for u in qkv_units(0, pair_tiles):
                u()
            late_loads()
            acc = None

            for r in range(3):
                qt, kt_t, vt_t, vaug0, vaug1 = pair_tiles
                vaug = (vaug0, vaug1)
                if r < 2:
                    next_tiles = alloc_pair_tiles(r + 1)
                    fillers = list(qkv_units(r + 1, next_tiles))
                else:
                    next_tiles, fillers = [], []
                    acc = big.tile([P, T // P, C], BF, tag="xt", name="acc")
                n_iters = sum(4 * qr + 4 for qr in range(nq))
                fill_i, iter_i = 0, 0
                half_n = max(1, nq - nq // 2)
                l_buf = sbp.tile([64, half_n * 2, 512], F32, tag="lbuf",
                                 name=f"lb{r}a")

                # attention per q-range; one sc/pt tile covers both heads of
                # the pair per k-tile so psc bufs=2 gives true lookahead
                for qr in range(nq):
                    n_k = 4 * qr + 4       # causal: k tiles 0..4qr+3
                    oT0 = pso.tile([P, 512], F32, tag="oT")
                    oT1 = pso.tile([P, 512], F32, tag="oT")
                    oT = (oT0, oT1)
                    for kti in range(n_k):
                        m = kti - 4 * qr
                        lo = 128 * max(0, m)  # causal col offset
                        sc = psc.tile([P, 2, 512], F32, tag="sc")
                        for a in (0, 1):
                            nc.tensor.matmul(
                                sc[:, a, lo:512],
                                kt_t[64 * a:64 * a + 64, ts(kti, P)],
                                qt[64 * a:64 * a + 64,
                                   qr * 512 + lo:(qr + 1) * 512],
                                start=True, stop=True)
                        iter_i += 1
                        while (fill_i < len(fillers)
                               and fill_i * n_iters < iter_i * len(fillers)):
                            fillers[fill_i]()
                            fill_i += 1
                        pt = ptp.tile([P, 2, 512], BF, tag="pt")
                        nc.scalar.activation(pt[:, :, lo:512],
                                             sc[:, :, lo:512], EXP)
                        if 0 <= m < 4:
                            for a in (0, 1):
                                blk = pt[:, a, 128 * m:128 * (m + 1)]
                                nc.vector.tensor_mul(blk, blk, tri[:])
                        for a in (0, 1):
                            nc.tensor.matmul(
                                oT[a][:, lo:512],
                                vaug[a][:, kti, :],
                                pt[:, a, lo:512],
                                start=(kti == 0), stop=(kti == n_k - 1))
                    # drain oT out of PSUM fast (normalization is deferred
                    # and batched so the slow reciprocal never holds slots)
                    for a in (0, 1):
                        nc.vector.tensor_copy(
                            att_all[:, 2 * r + a, ts(qr, 512)],
                            oT[a][0:64, :])
                        nc.vector.tensor_copy(
                            l_buf[:, 2 * (qr % half_n) + a, :],
                            oT[a][64:128, :])

                    if qr == nq - 1 or qr == nq // 2 - 1:
                        # half boundary: batched reciprocal + in-place
                        # normalize, then ship this half's exchange
                        h = 0 if qr == nq // 2 - 1 else (0 if nq < 2 else 1)
                        q0 = 0 if h == 0 else nq // 2
                        nslots = 2 * (qr - q0 + 1)
                        rb = sbp.tile([64, half_n * 2, 512], F32, tag="rb",
                                      name=f"rb{r}_{qr}")
                        nc.vector.reciprocal(rb[:, 0:nslots, :],
                                             l_buf[:, 0:nslots, :])
                        for qq in range(q0, qr + 1):
                            for a in (0, 1):
                                sl = att_all[:, 2 * r + a, ts(qq, 512)]
                                nc.vector.tensor_mul(
                                    sl, sl,
                                    rb[:, 2 * (qq % half_n) + a, :])
                        emit_ag_half(r, h)
                        if nq < 2:
                            emit_ag_half(r, 1)
                        if r == 2:
                            # own-half projection for this half's t-chunks
                            # doubles as PE filler for the last pair
                            for qq in range(q0, qr + 1):
                                for tch in range(4 * qq,
                                                 min(4 * qq + 4, T // P)):
                                    projA_unit(tch, acc)()
                        if h == 0 and nq >= 2:
                            l_buf = sbp.tile([64, half_n * 2, 512], F32,
                                             tag="lbuf", name=f"lb{r}b")

                while fill_i < len(fillers):
                    fillers[fill_i]()
                    fill_i += 1
                pair_tiles = next_tiles

            # ---- output projection pass B: partner half + bias ----
            for tch in range(T // P):
                po = psc.tile([P, C], F32, tag="sc")
                for j in range(6):
                    lhsT = att_oth[:, j, ts(tch, P)]
                    for (c0, c1) in co_ranges:
                        nc.tensor.matmul(po[:, c0:c1], lhsT,
                                         wpt[:, 6 + j, c0:c1],
                                         start=(j == 0), stop=False)
                for (c0, c1) in co_ranges:
                    nc.tensor.matmul(po[:, c0:c1], ones1[:],
                                     bp_sb[:, c0:c1], start=False, stop=True)
                ot = sbp.tile([P, C], F32, tag="out")
                nc.vector.tensor_add(ot[:], po[:], acc[:, tch, :])
                nc.sync.dma_start(out_d[ts(tch, P), :], ot[:])

    nc.compile()
    return nc


_cached = {}


def get_nc(T=T_FULL):
    if T not in _cached:
        _cached[T] = build(T)
    return _cached[T]


def _make_in_maps(x, Wq, Wk, Wv, Wp, bp):
    scale = HS ** -0.5
    in_maps = []
    for c in range(N_CORES):
        b, s = c // 2, c % 2
        heads = list(range(6 * s, 6 * s + 6))
        xt = np.ascontiguousarray(np.asarray(x)[b].T).astype(bf16)
        def packw(W, sc=1.0):
            cols = []
            for rr in range(3):
                h0, h1 = heads[2 * rr], heads[2 * rr + 1]
                cols.append(np.concatenate([W[h0], W[h1]], axis=1))
            return (np.concatenate(cols, axis=1) * sc).astype(bf16)
        order = heads + [h for h in range(H) if h not in heads]
        wpt = np.concatenate(
            [np.asarray(Wp)[:, 64 * h:64 * h + 64].T for h in order],
            axis=0).astype(bf16)
        meta = ((c ^ 1) * 128 + 2 * np.arange(64, dtype=np.int32)[:, None]
                + np.arange(2, dtype=np.int32)[None, :]).astype(np.int32)
        in_maps.append({
            "xt": xt,
            "wq": packw(np.asarray(Wq), scale),
            "wk": packw(np.asarray(Wk)),
            "wv": packw(np.asarray(Wv)),
            "wpt": wpt,
            "bp": np.asarray(bp).reshape(1, C).astype(bf16),
            "meta": meta,
        })
    return in_maps


def kernel(x, Wq, Wk, Wv, Wp, bp):
    nc = get_nc(T_FULL)
    in_maps = _make_in_maps(x, Wq, Wk, Wv, Wp, bp)
    res = run_bass_kernel_spmd(nc, in_maps, list(range(N_CORES)))
    out = np.stack([res.results[2 * b]["out"] for b in range(B)])
    return out.astype(np.float32)
